# revision 10
# baseline (speedup 1.0000x reference)
"""Inverse separable wavelet synthesis (stride-2 transposed conv, 9 taps,
36 -> 12 -> 4 channels, 256x256 -> 512x512) on 8 trn2 NeuronCores.

Formulation: both passes are expressed as dense matmuls against three
host-precomputed banded operator matrices A_beta [256 in, 512 out] (one per
wavelet band), with symmetric padding + border-mask sign folded into the
operators.  H == W == 256 so the X-pass and Y-pass share the same operators.

    t[b,h,m,g]    = sum_{beta,j} A_beta[j,m] * x[b,h,j,3g+beta]      (X pass)
    out[b,n,m,q]  = sum_{beta,i} A_beta[i,n] * t[b,i,m,3q+beta]      (Y pass)

Per-core layout (pure batch parallelism, 2 images per core):
  stage A : lhsT = operator block [128 w-in, 128 w2-out], moving = x arranged
            [w-in partitions, (group, h) free] -> PSUM [w2, (g, h)]
  PE transpose 128x128 tiles: t1 [w2, (g,h)] -> t2 [h, (g, w2)]
  stage B : lhsT = operator block [128 h-in, 128 h2-out], moving = t2
            [h-in partitions, (g2, w2) free] -> PSUM [h2, (g2, w2)]
Matmuls run in float32r (full-rate fp32 PE mode), accumulation in fp32 PSUM.
"""

import numpy as np
from contextlib import ExitStack

import concourse.bass as bass
import concourse.bacc as bacc
import concourse.mybir as mybir
import concourse.tile as tile
from concourse.bass_utils import run_bass_kernel_spmd

B, H, W, C = 16, 256, 256, 36
NCORES = 8
BPC = B // NCORES  # batches per core
W2 = 2 * W
H2 = 2 * H
F32 = mybir.dt.float32
F32R = mybir.dt.float32r
BF16 = mybir.dt.bfloat16

SMOOTH = [0.0, 0.0, 1.0 / 16.0, 0.5, 14.0 / 16.0, 0.5, 1.0 / 16.0, 0.0, 0.0]
EVEN = [-1.0 / 128.0, -1.0 / 16.0, -10.0 / 64.0, -7.0 / 16.0, 85.0 / 64.0,
        -7.0 / 16.0, -10.0 / 64.0, -1.0 / 16.0, -1.0 / 128.0]
ODD = [1.0 / 256.0, 1.0 / 32.0, 15.0 / 128.0, 17.0 / 32.0, 0.0,
       -17.0 / 32.0, -15.0 / 128.0, -1.0 / 32.0, -1.0 / 256.0]

# Which 128-row k-tiles of the operator feed each 128-col output block:
# out block n covers in rows [64n-2, 64n+65] (+ folded reflections).
KTS = {0: (0,), 1: (0, 1), 2: (0, 1), 3: (1,)}


def _build_operator_array():
    """[3 bands, 2 ktiles, 128 in-rows, 512 out-cols] fp32 operator."""
    inv = np.array([SMOOTH, EVEN, ODD], dtype=np.float64)
    S = 256
    Sp = S + 6
    j = np.arange(Sp)[:, None]
    m = np.arange(2 * S)[None, :]
    t = m + 10 - 2 * j
    valid = (t >= 0) & (t <= 8)
    P = np.zeros((3, Sp, 2 * S))
    for b in range(3):
        P[b][valid] = inv[b][t[valid]]
    # border mask: odd band negated on the 3-wide padded border
    P[2, [0, 1, 2, Sp - 3, Sp - 2, Sp - 1], :] *= -1.0
    # fold symmetric padding: pad[0..2] = x[2],x[1],x[0]; pad[-3:] = x[-1],x[-2],x[-3]
    A = P[:, 3:3 + S].copy()
    A[:, 2] += P[:, 0]
    A[:, 1] += P[:, 1]
    A[:, 0] += P[:, 2]
    A[:, S - 1] += P[:, Sp - 3]
    A[:, S - 2] += P[:, Sp - 2]
    A[:, S - 3] += P[:, Sp - 1]
    return np.ascontiguousarray(A.reshape(3, 2, 128, 2 * S).astype(np.float32))


MODE = "bf16"  # "bf16" or "f32r"


def _build_program(repeat=1, mode=None):
    mode = mode or MODE
    DT = BF16 if mode == "bf16" else F32R
    nc = bacc.Bacc("TRN2", target_bir_lowering=False)
    x = nc.declare_dram_parameter("x", [BPC, H, W, C], DT, isOutput=False)
    a_op = nc.declare_dram_parameter("a_op", [3, 2, 128, W2], DT, isOutput=False)
    ident = nc.declare_dram_parameter("ident", [128, 128], DT, isOutput=False)
    out = nc.declare_dram_parameter("out", [BPC, H2, W2, 4], F32, isOutput=True)

    with tile.TileContext(nc) as tc, ExitStack() as ctx:
        const = ctx.enter_context(tc.tile_pool(name="const", bufs=1))
        xpool = ctx.enter_context(tc.tile_pool(name="xp", bufs=4))
        t1pool = ctx.enter_context(tc.tile_pool(name="t1p", bufs=2))
        t2pool = ctx.enter_context(tc.tile_pool(name="t2p", bufs=1))
        opool = ctx.enter_context(tc.tile_pool(name="op", bufs=2))
        psA = ctx.enter_context(tc.tile_pool(name="psA", bufs=3, space="PSUM"))
        psT = ctx.enter_context(tc.tile_pool(name="psT", bufs=2, space="PSUM"))
        psB = ctx.enter_context(tc.tile_pool(name="psB", bufs=2, space="PSUM"))

        a_sb = {}
        for beta in range(3):
            for kt in range(2):
                at = const.tile([128, W2], DT, name=f"a_{beta}_{kt}",
                                tag=f"a_{beta}_{kt}")
                nc.sync.dma_start(at[:], a_op[beta, kt])
                a_sb[beta, kt] = at
        ident_sb = const.tile([128, 128], DT, name="ident_sb", tag="ident")
        nc.sync.dma_start(ident_sb[:], ident[:])

        for rep in range(repeat):
          for b_ in range(BPC):
            b = b_
            rb = rep * BPC + b_
            # x tiles: [w partitions, (h, c) free] per (w-tile, h-chunk)
            xt = {}
            for wt in range(2):
                for hc in range(2):
                    xtile = xpool.tile([128, 128 * C], DT,
                                       name=f"x_{rb}_{wt}_{hc}", tag="x")
                    src = x[b, hc * 128:(hc + 1) * 128,
                            wt * 128:(wt + 1) * 128, :].rearrange(
                                "h w c -> w h c")
                    nc.sync.dma_start(
                        xtile.rearrange("w (h c) -> w h c", c=C), src)
                    xt[wt, hc] = xtile

            t2 = {}
            for hc in range(2):
                t2[hc] = t2pool.tile([128, 12 * W2], DT,
                                     name=f"t2_{rb}_{hc}", tag=f"t2_{hc}")

            for blk in range(4):
                t1 = t1pool.tile([128, 12 * H], DT, name=f"t1_{rb}_{blk}",
                                 tag="t1")
                t1v = t1.rearrange("p (g h) -> p g h", g=12)
                # stage A: accumulate 3 bands (x 1-2 k-tiles) into PSUM
                for hc in range(2):
                    for gq in range(3):
                        ps = psA.tile([128, 512], F32,
                                      name=f"psA_{rb}_{blk}_{hc}_{gq}",
                                      tag="psA")
                        mms = [(beta, kt) for beta in range(3)
                               for kt in KTS[blk]]
                        for i, (beta, kt) in enumerate(mms):
                            lhsT = a_sb[beta, kt][
                                :, blk * 128:(blk + 1) * 128]
                            rhs = xt[kt, hc].rearrange(
                                "w (h g c) -> w c g h", g=12, c=3)[
                                    :, beta, gq * 4:(gq + 1) * 4, :]
                            nc.tensor.matmul(ps[:], lhsT, rhs,
                                             start=(i == 0),
                                             stop=(i == len(mms) - 1))
                        nc.vector.tensor_copy(
                            out=t1v[:, gq * 4:(gq + 1) * 4,
                                    hc * 128:(hc + 1) * 128],
                            in_=ps.rearrange("p (g h) -> p g h", g=4))
                # transpose t1 [w2, h] tiles into t2 [h, w2]
                for g in range(12):
                    for hc in range(2):
                        pt = psT.tile([128, 128], DT,
                                      name=f"psT_{rb}_{blk}_{g}_{hc}",
                                      tag="psT")
                        nc.tensor.transpose(
                            pt[:],
                            t1v[:, g, hc * 128:(hc + 1) * 128],
                            ident_sb[:])
                        dst = t2[hc][:, g * W2 + blk * 128:
                                     g * W2 + (blk + 1) * 128]
                        if (g + hc) % 2 == 0:
                            nc.vector.tensor_copy(out=dst, in_=pt[:])
                        else:
                            nc.scalar.copy(out=dst, in_=pt[:])

            # stage B
            for h2 in range(4):
                osb = opool.tile([128, W2 * 4], F32, name=f"osb_{rb}_{h2}",
                                 tag="osb")
                osbv = osb.rearrange("p (w c) -> p c w", c=4)
                for wc in range(4):
                    ps = psB.tile([128, 512], F32,
                                  name=f"psB_{rb}_{h2}_{wc}", tag="psB")
                    mms = [(beta, kt) for beta in range(3) for kt in KTS[h2]]
                    for i, (beta, kt) in enumerate(mms):
                        lhsT = a_sb[beta, kt][
                            :, h2 * 128:(h2 + 1) * 128]
                        rhs = t2[kt].rearrange(
                            "h (q c w) -> h c q w", q=4, c=3)[
                                :, beta, :, wc * 128:(wc + 1) * 128]
                        nc.tensor.matmul(ps[:], lhsT, rhs,
                                         start=(i == 0),
                                         stop=(i == len(mms) - 1))
                    nc.scalar.copy(
                        out=osbv[:, :, wc * 128:(wc + 1) * 128],
                        in_=ps.rearrange("p (c w) -> p c w", c=4))
                dst = out[b, h2 * 128:(h2 + 1) * 128, :, :].rearrange(
                    "h w c -> h (w c)")
                nc.sync.dma_start(dst, osb[:])
    nc.compile()
    return nc


def _round_fp32r(x):
    """Round fp32 array to fp32r (fp32 with 11-bit mantissa, RNE) on host."""
    b = x.view(np.uint32).astype(np.uint64)
    b = (b + 0x7FF + ((b >> 12) & 1)) & ~np.uint64(0xFFF)
    return b.astype(np.uint32).view(np.float32)


_PROGRAMS = {}


def _get_program(repeat=1, mode=None):
    mode = mode or MODE
    key = (repeat, mode)
    if key not in _PROGRAMS:
        _PROGRAMS[key] = _build_program(repeat, mode)
    return _PROGRAMS[key]


def _run(inputs, trace=False, tmpdir=None, repeat=1, mode=None):
    """Returns (full output [16,512,512,4], BassKernelResults)."""
    import ml_dtypes
    mode = mode or MODE
    inputs = np.ascontiguousarray(np.asarray(inputs, dtype=np.float32))
    assert inputs.shape == (B, H, W, C), inputs.shape
    nc = _get_program(repeat, mode)
    if mode == "bf16":
        a4 = _build_operator_array().astype(ml_dtypes.bfloat16)
        identity = np.ascontiguousarray(np.eye(128, dtype=ml_dtypes.bfloat16))
        shards = inputs.astype(ml_dtypes.bfloat16).reshape(NCORES, BPC, H, W, C)
    else:
        a4 = _build_operator_array()
        identity = np.ascontiguousarray(np.eye(128, dtype=np.float32))
        shards = _round_fp32r(inputs).reshape(NCORES, BPC, H, W, C)
    in_maps = [{"x": np.ascontiguousarray(shards[c]), "a_op": a4,
                "ident": identity} for c in range(NCORES)]
    res = run_bass_kernel_spmd(nc, in_maps, core_ids=list(range(NCORES)),
                               trace=trace, tmpdir=tmpdir)
    outs = [np.asarray(res.results[c]["out"]) for c in range(NCORES)]
    full = np.concatenate(outs, axis=0).astype(np.float32)
    return full, res


def kernel(inputs):
    full, _ = _run(inputs)
    return full
